# revision 40
# baseline (speedup 1.0000x reference)
"""Distributed single-head attention for TRN2 (8 NeuronCores).

Reference computation (per batch b):
    q = x @ Wq; k = x @ Wk; v = x @ Wv          (x: [S, E])
    s = (q @ k.T) / sqrt(DK) - 1e15 * mask
    out = softmax(s, axis=-1) @ v               ([S, DV])

Sharding: 8 cores = 4 batches x 2 sequence halves. Each core computes
attention for 1024 queries of one batch; K/V are recomputed per core from
the full sequence (softmax is permutation invariant over keys, so the
sequence is rotated per-core to put its queries first).

Host-prepared per-core layout:
  - xt  [E, S]  bf16: x_b^T, sequence permuted, queries first.
  - wq/wk/wv [P, EC*DK] bf16: packed [p, c*DK+d] = W[c*128+p, d]
                (wq pre-scaled by 1/sqrt(DK)).
  - mm_ [P, KTILES*SQ] fp8e4: the RAW mask ({0,1} exact in fp8 — halves
                mask DMA), keys permuted like xt, partition-major
                (mm_[p, t*SQ+q] = mask[key t*128+p, q]) for sequential DRAM.
  - pen [P, P] fp8e4: -30 * I. The mask penalty is applied INSIDE the
                score accumulation group: st = kt^T qt - 30*m, so
                exp(st) is already-masked P (exp(-30) ~ 5e-12 is
                negligible vs softmax sums ~1e3). This removes the wide
                DVE mask multiply — DVE ops pay a ~(dur-266ns) pipeline
                DRAIN before the next op can issue, so each [128,2048]
                op costs ~1.6us effective; DVE was the loop pacer.
  - out [DV, SQ] bf16: attention output transposed; host casts + transposes.

Schedule (v4):
  - PE warmup: dummy matmuls on ones right after the preamble trip the
    HAM activity monitor so the PE clock is 2.4 GHz when chunk 0 lands.
  - sync-queue DMAs (16 engines; scalar queue only has 4), first-use
    order: w, xa chunks (query-half columns), xb ~2 behind, masks last —
    same-queue FIFO delays mask traffic behind the x stream for free.
  - chunk loop c=0..7: qt0 qt1 k01 (xa), k23 (xb), vtA (xa) per chunk,
    into three [128,1024] PSUM pair-tiles + 2 vt banks. Wide PSUM->SBUF
    copies in first-use order: qt (ACT); k01, k23, vtA (DVE).
  - PSUM bank map is chosen so the score tiles alias the EARLIEST-copied
    projection banks: st pool (3 bufs, banks 2-7) reuses qt/k01/k23;
    the OT accumulators (2x [128,512]) reuse the vt banks 0-1 after the
    pre-loop vtB matmuls + copies. st bufs=3 decouples score(t+2) WAR
    from exp(t) so the exp stream never waits a just-issued score.
  - attention loop over PAIRS of key tiles: per tile, scores are 2 bf16
    matmuls + 2 fp8 penalty matmuls (same accumulation group), exp on
    ACT, then OT. DVE only accumulates the pair denominator (pairs 0-6).
  - denominator: ones[128,128]^T @ (folded pairs-0-6 sum, then pair-7 P)
    accumulated into an st-pool PSUM tile during pair 7 — rowsum(q)
    replicated across partitions; normalization is an elementwise
    multiply against the OT accumulators in native [dv, q] layout
    (reciprocal_approx_fast); the host transposes the [dv, q] result.
"""
import math
from contextlib import ExitStack

import ml_dtypes
import numpy as np

import concourse.bass as bass
import concourse.tile as tile
from concourse import bacc, bass_isa, masks, mybir
from concourse.bass_utils import run_bass_kernel_spmd

B, S, E, DK, DV = 4, 2048, 1024, 128, 128
SQ = S // 2  # queries per core
P = 128  # SBUF partitions
EC = E // P  # contraction chunks for projections
KTILES = S // P  # key tiles
N_WARM = 40  # PE warmup matmuls: bridge the PE from the preamble until
# the first x chunk lands (~11us) so the HAM clock gate never re-throttles
PEN = 30.0  # additive mask penalty (exact in fp8e4; exp(-30) ~ 5e-12)

f32 = mybir.dt.float32
bf16 = mybir.dt.bfloat16
f8e4 = mybir.dt.float8e4

# test.py pokes these to get profiling info
TRACE = False
LAST_RESULT = None


def build():
    nc = bacc.Bacc()
    xta = nc.declare_dram_parameter("xta", [E, SQ], bf16, isOutput=False)
    xtb = nc.declare_dram_parameter("xtb", [E, SQ], bf16, isOutput=False)
    wq = nc.declare_dram_parameter("wq", [P, EC * DK], bf16, isOutput=False)
    wk = nc.declare_dram_parameter("wk", [P, EC * DK], bf16, isOutput=False)
    wv = nc.declare_dram_parameter("wv", [P, EC * DV], bf16, isOutput=False)
    mm_ = nc.declare_dram_parameter("mm", [P, KTILES * SQ], f8e4, isOutput=False)
    pen = nc.declare_dram_parameter("pen", [P, P], f8e4, isOutput=False)
    out = nc.declare_dram_parameter("out", [P, SQ], bf16, isOutput=True)

    with ExitStack() as ctx:
        tc = ctx.enter_context(tile.TileContext(nc))
        const_pool = ctx.enter_context(tc.tile_pool(name="const", bufs=1))
        in_pool = ctx.enter_context(tc.tile_pool(name="inputs", bufs=1))
        proj_sb = ctx.enter_context(tc.tile_pool(name="proj", bufs=1))
        p_pool = ctx.enter_context(tc.tile_pool(name="p", bufs=6))
        stat = ctx.enter_context(tc.tile_pool(name="stat", bufs=1))

        # vt PSUM pool (banks 0-1): vtA -> vtB -> the two OT accumulators
        vt_ps = ctx.enter_context(tc.tile_pool(name="vt_ps", bufs=2, space="PSUM"))

        ones_mat = const_pool.tile([P, P], bf16)
        nc.gpsimd.memset(ones_mat[:], 1.0)
        # preload the exp table set off the critical path
        warm = const_pool.tile([1, 2], f32)
        nc.gpsimd.memset(warm[:], 0.0)
        nc.scalar.activation(warm[:], warm[:], mybir.ActivationFunctionType.Exp)

        # --- input loads: sync HWDGE queue, first-use order ---
        w_sb = {}
        for name in ("wq", "wk", "wv"):
            w_sb[name] = in_pool.tile([P, EC * DK], bf16, tag=name, name=f"w_{name}")
        pen_sb = in_pool.tile([P, P], f8e4, tag="pen", name="pen")
        xa_sb = []
        xb_sb = []
        for c in range(EC):
            xa_sb.append(in_pool.tile([P, SQ], bf16, tag=f"xa{c}", name=f"xa{c}"))
            xb_sb.append(in_pool.tile([P, SQ], bf16, tag=f"xb{c}", name=f"xb{c}"))
        m_all = [
            in_pool.tile([P, 4 * SQ], f8e4, tag=f"mq{qtr}", name=f"mq{qtr}")
            for qtr in range(4)
        ]

        # ALL xa chunks stream before any xb: the xa-only slabs
        # (qt/k01/vtA) then finish ~6us earlier, so the qt/k01 copies,
        # the first scores, and the exp stream all start that much
        # sooner. k23 (xb-only) runs as its own back-to-back PE stream
        # behind the xa one. mask quarter 0 slots in before xb6/xb7
        # (which only feed the chunk-7 k23 matmuls).
        # Single sync HWDGE queue (16 DMA engines — scalar's only gets 4,
        # and a parallel mask queue steals xa bandwidth). xta/xtb are
        # separate contiguous DRAM tensors so every transfer is a fully
        # sequential read. FIFO insertion points: mask quarter 0 after
        # xb4 (pen01 needs it right after the qt/k01 copies), the rest
        # behind the xb tail (their pen matmuls run pairs 0+).
        nc.sync.dma_start(w_sb["wq"][:], wq[:, :])
        nc.sync.dma_start(xa_sb[0][:], xta[0:P, :])
        nc.sync.dma_start(w_sb["wk"][:], wk[:, :])
        nc.sync.dma_start(w_sb["wv"][:], wv[:, :])
        nc.sync.dma_start(pen_sb[:], pen[:, :])
        for c in range(1, EC):
            nc.sync.dma_start(xa_sb[c][:], xta[c * P : (c + 1) * P, :])
        # mask quarters 0-1 ahead of the xb stream (their pen matmuls run
        # right after the qt/k01 copies); quarters 2-3 behind it (their
        # consumers run from loop pair 1 on)
        nc.sync.dma_start(m_all[0][:], mm_[:, 0 : 4 * SQ])
        nc.sync.dma_start(m_all[1][:], mm_[:, 4 * SQ : 8 * SQ])
        for c in range(EC):
            nc.sync.dma_start(xb_sb[c][:], xtb[c * P : (c + 1) * P, :])
        for qtr in range(2, 4):
            nc.sync.dma_start(
                m_all[qtr][:], mm_[:, qtr * 4 * SQ : (qtr + 1) * 4 * SQ]
            )

        def m_tile(t):  # mask columns for key tile t
            qtr, o = divmod(t, 4)
            return m_all[qtr][:, o * SQ : (o + 1) * SQ]

        # --- PE warmup: release the HAM clock gate before chunk 0 lands ---
        with tc.tile_pool(name="warm_ps", bufs=1, space="PSUM") as warm_pool:
            wps = warm_pool.tile([P, P], f32)
            for i in range(N_WARM):
                nc.tensor.matmul(
                    wps[:], ones_mat[:], ones_mat[:],
                    start=(i == 0), stop=(i == N_WARM - 1),
                )

        # running sum over PAIRS of masked P tiles (pairs 0..6)
        acc_d = stat.tile([P, 2 * SQ], bf16)

        # --- projections ---
        qt_sb = proj_sb.tile([P, SQ], bf16)
        kt_sb = proj_sb.tile([P, 4 * 512], bf16)  # 4 slabs, contiguous

        def kt_slab(s):
            return kt_sb[:, s * 512 : (s + 1) * 512]

        vt_sb = [
            proj_sb.tile([P, SQ], bf16, tag=f"vt{h}", name=f"vt{h}")
            for h in range(2)
        ]
        v_sb = [
            proj_sb.tile([P, 8 * DV], bf16, tag=f"v{h}", name=f"v{h}")
            for h in range(2)
        ]

        def v_tile(t):
            return v_sb[t // 8][:, (t % 8) * DV : (t % 8 + 1) * DV]

        def wslice(w, c):
            return w[:, c * DK : (c + 1) * DK]

        def xs(c, j):  # x chunk c, 512-column slab j (0..3)
            half = xa_sb if j < 2 else xb_sb
            jj = j % 2
            return half[c][:, jj * 512 : (jj + 1) * 512]

        ps_vt = {}
        for tag in ("vtA0", "vtA1"):
            ps_vt[tag] = vt_ps.tile([P, 512], f32, tag="vtps", name=f"ps_{tag}")

        proj_ctx = ExitStack()
        proj_ps = proj_ctx.enter_context(
            tc.tile_pool(name="proj_ps", bufs=2, space="PSUM")
        )
        ps_qt = proj_ps.tile([P, SQ], f32, tag="pps", name="ps_qt")
        ps_k01 = proj_ps.tile([P, SQ], f32, tag="pps", name="ps_k01")

        # xa-only stream: qt/k01/vtA accumulate over the xa chunks alone,
        # so they never wait on the trailing xb transfers. k23 (xb-only)
        # runs later through the vt banks, after the vtA copies.
        for c in range(EC):
            st_flags = dict(start=(c == 0), stop=(c == EC - 1))
            for j in range(2):  # QT (queries = columns 0..1023)
                nc.tensor.matmul(
                    ps_qt[:, j * 512 : (j + 1) * 512],
                    wslice(w_sb["wq"], c), xs(c, j), **st_flags,
                )
            for j in range(2):  # KT slabs 0-1 (keys 0..1023, xa)
                nc.tensor.matmul(
                    ps_k01[:, j * 512 : (j + 1) * 512],
                    wslice(w_sb["wk"], c), xs(c, j), **st_flags,
                )
            for j in range(2):  # VT slabs A0-A1 (keys 0..1023, xa)
                nc.tensor.matmul(
                    ps_vt[f"vtA{j}"][:], wslice(w_sb["wv"], c), xs(c, j), **st_flags
                )

        # wide PSUM -> SBUF copies, first-use order per engine
        nc.scalar.copy(qt_sb[:], ps_qt[:])
        nc.vector.tensor_copy(kt_sb[:, 0:1024], ps_k01[:])
        nc.vector.tensor_copy(vt_sb[0][:, 0:512], ps_vt["vtA0"][:])
        nc.vector.tensor_copy(vt_sb[0][:, 512:1024], ps_vt["vtA1"][:])

        # V transpose A: ONE grouped xbar op; out[k, t, dv] = vt[dv, t*128+k]
        nc.sync.dma_start_transpose(
            v_sb[0][:].rearrange("p (t d) -> p t d", t=8), vt_sb[0][:]
        )

        proj_ctx.close()  # qt/k01 banks (+warm's) -> st pool

        st_ctx = ctx.enter_context(ExitStack())
        st_ps = st_ctx.enter_context(
            tc.tile_pool(name="st_ps", bufs=3, space="PSUM")
        )

        st_tiles = []

        def score_mms(t):
            st = st_ps.tile([P, SQ], f32, tag="st", name=f"st{t}")
            st_tiles.append(st)
            s, o = divmod(t, 4)
            for j in range(2):
                nc.tensor.matmul(
                    st[:, j * 512 : (j + 1) * 512],
                    kt_slab(s)[:, o * P : (o + 1) * P],
                    qt_sb[:, j * 512 : (j + 1) * 512],
                    start=True,
                    stop=False,
                )
            for j in range(2):  # -30 * mask, same accumulation group (fp8)
                nc.tensor.matmul(
                    st[:, j * 512 : (j + 1) * 512],
                    pen_sb[:],
                    m_tile(t)[:, j * 512 : (j + 1) * 512],
                    start=False,
                    stop=True,
                )

        # PE order from here: sc/pen(0-5) -> k23 xb-stream -> sc/pen(6-7)
        # -> vtB xb-stream -> loop (lead 8). Tiles 0-7 only need the
        # qt/k01 copies + mask quarters 0-1, so the ACT exp stream runs
        # uninterrupted while the xb-side work (paced by xb DMA-completion
        # semaphores that lag the data ~2.5us) fills the PE gaps behind
        # the pre-emitted scores.
        for t in range(6):
            score_mms(t)

        # k23 through the vt banks (WAR on the vtA copies)
        ps_k23 = [
            vt_ps.tile([P, 512], f32, tag="vtps", name=f"ps_k23{j}")
            for j in range(2)
        ]
        for c in range(EC):
            st_flags = dict(start=(c == 0), stop=(c == EC - 1))
            for j in range(2):  # KT slabs 2-3 (keys 1024..2047, xb)
                nc.tensor.matmul(
                    ps_k23[j][:], wslice(w_sb["wk"], c), xs(c, 2 + j), **st_flags
                )
        nc.vector.tensor_copy(kt_sb[:, 1024:1536], ps_k23[0][:])
        nc.vector.tensor_copy(kt_sb[:, 1536:2048], ps_k23[1][:])

        for t in range(6, 8):
            score_mms(t)

        # VT slabs B0/B1 on the vt banks (WAR on the k23 copies); their
        # copies + per-slab transposes are emitted inside loop pairs 1-2
        # so the DVE accumulates for pairs 0-1 aren't delayed.
        ps_vt["vtB0"] = vt_ps.tile([P, 512], f32, tag="vtps", name="ps_vtB0")
        ps_vt["vtB1"] = vt_ps.tile([P, 512], f32, tag="vtps", name="ps_vtB1")
        for c in range(EC):
            for j in range(2):
                nc.tensor.matmul(
                    ps_vt[f"vtB{j}"][:],
                    wslice(w_sb["wv"], c),
                    xs(c, 2 + j),
                    start=(c == 0),
                    stop=(c == EC - 1),
                )

        # OT accumulators on the vt banks (freed by the vtB copies)
        ot = [
            vt_ps.tile([P, 512], f32, tag="vtps", name=f"ot{j}") for j in range(2)
        ]

        rs = None  # denominator PSUM tile (st pool), created at pair 6
        last_pp = None

        for r in range(KTILES // 2):
            pp = p_pool.tile([P, 2 * SQ], bf16, tag="p", name=f"pp{r}")
            for h in range(2):
                t = 2 * r + h
                nc.scalar.activation(
                    pp[:, h * SQ : (h + 1) * SQ],
                    st_tiles[t][:],
                    mybir.ActivationFunctionType.Exp,
                )
            # accumulate pairs 0..6; pair 7 feeds the denominator matmuls
            if r == 0:
                nc.vector.tensor_copy(acc_d[:], pp[:])
            elif r < 7:
                nc.vector.tensor_add(acc_d[:], acc_d[:], pp[:])
            else:
                last_pp = pp
                # finish the denominator BEFORE the final OTs so the
                # recip -> normalize -> store tail starts earlier
                for h in range(2):
                    for j in range(2):
                        nc.tensor.matmul(
                            rs[:, j * 512 : (j + 1) * 512], ones_mat[:],
                            pp[:, h * SQ + j * 512 : h * SQ + (j + 1) * 512],
                            start=False, stop=(h == 1),
                        )
            for h in range(2):
                t = 2 * r + h
                if t + 8 < KTILES:
                    score_mms(t + 8)
                for j in range(2):
                    nc.tensor.matmul(
                        ot[j][:],
                        v_tile(t),
                        pp[:, h * SQ + j * 512 : h * SQ + (j + 1) * 512],
                        start=(t == 0),
                        stop=(t == KTILES - 1),
                    )
            if r == 1:  # vtB0 copy + transpose (v tiles 8-11), DVE stream
                nc.vector.tensor_copy(vt_sb[1][:, 0:512], ps_vt["vtB0"][:])
                nc.sync.dma_start_transpose(
                    v_sb[1][:, 0:512].rearrange("p (t d) -> p t d", t=4),
                    vt_sb[1][:, 0:512],
                )
            if r == 2:  # vtB1 copy + transpose (v tiles 12-15)
                nc.vector.tensor_copy(vt_sb[1][:, 512:1024], ps_vt["vtB1"][:])
                nc.sync.dma_start_transpose(
                    v_sb[1][:, 512:1024].rearrange("p (t d) -> p t d", t=4),
                    vt_sb[1][:, 512:1024],
                )
            if r == 6:
                # fold pairs 0..6 and start the denominator matmuls on an
                # st-pool tile (WAR: its bank tenant exp13 just ran)
                acc_f = stat.tile([P, SQ], bf16)
                nc.vector.tensor_add(
                    acc_f[:], acc_d[:, 0:SQ], acc_d[:, SQ : 2 * SQ]
                )
                rs = st_ps.tile([P, SQ], f32, tag="st", name="rs")
                for j in range(2):
                    nc.tensor.matmul(
                        rs[:, j * 512 : (j + 1) * 512], ones_mat[:],
                        acc_f[:, j * 512 : (j + 1) * 512],
                        start=True, stop=False,
                    )

        # --- epilogue: normalize in [dv, q] layout, store ---
        rcp_rep = stat.tile([P, SQ], f32)
        o_sb = stat.tile([P, SQ], bf16)
        for j in range(2):  # per-half pipeline: recip -> scale -> store
            sl = slice(j * 512, (j + 1) * 512)
            nc.vector.reciprocal_approx_fast(rcp_rep[:, sl], rs[:, sl])
            nc.vector.tensor_mul(o_sb[:, sl], ot[j][:], rcp_rep[:, sl])
            nc.sync.dma_start(out[:, sl], o_sb[:, sl])

    nc.compile()
    return nc


_NC_CACHE = None


def kernel(inputs, mask, Wq, Wk, Wv):
    global _NC_CACHE, LAST_RESULT
    inputs = np.asarray(inputs)
    mask = np.asarray(mask)
    bf = ml_dtypes.bfloat16
    f8 = ml_dtypes.float8_e4m3fn
    scale = np.float32(1.0 / math.sqrt(DK))

    def pack_w(w):  # [E, DK] -> [p, c*DK+d] = w[c*128+p, d]
        w = np.asarray(w).astype(bf)
        return np.ascontiguousarray(
            w.reshape(EC, P, DK).transpose(1, 0, 2).reshape(P, EC * DK)
        )

    wq_h = pack_w(np.asarray(Wq) * scale)
    wk_h = pack_w(Wk)
    wv_h = pack_w(Wv)
    pen_h = (-PEN * np.eye(P, dtype=np.float32)).astype(f8)

    if _NC_CACHE is None:
        _NC_CACHE = build()
    nc = _NC_CACHE

    m8 = mask.astype(f8)  # raw mask, {0,1} exact in fp8
    in_maps = []
    for core in range(8):
        b, h = divmod(core, 2)
        q0 = h * SQ
        idx = np.r_[q0:S, 0:q0]  # rotate so this core's queries come first
        xb = inputs[b]  # [S, E] f32
        xt_core = np.asarray(xb[idx].T).astype(bf)  # [E, S]
        xta_core = np.ascontiguousarray(xt_core[:, 0:SQ])
        xtb_core = np.ascontiguousarray(xt_core[:, SQ:S])
        mt = m8[b, q0 : q0 + SQ, :][:, idx].T  # [S, SQ] keys-major
        # partition-major swizzle: mm_core[p, t*SQ+q] = mt[t*128+p, q]
        mm_core = np.ascontiguousarray(
            mt.reshape(KTILES, P, SQ).transpose(1, 0, 2).reshape(P, KTILES * SQ)
        )
        in_maps.append(
            {
                "xta": xta_core, "xtb": xtb_core,
                "wq": wq_h, "wk": wk_h, "wv": wv_h,
                "mm": mm_core, "pen": pen_h,
            }
        )

    res = run_bass_kernel_spmd(nc, in_maps, list(range(8)), trace=TRACE)
    LAST_RESULT = res
    outp = np.empty((B, S, DV), np.float32)
    for core in range(8):
        b, h = divmod(core, 2)
        q0 = h * SQ
        o = np.asarray(res.results[core]["out"]).astype(np.float32)  # [DV, SQ]
        outp[b, q0 : q0 + SQ, :] = o.T
    return outp


# revision 43
# speedup vs baseline: 1.0135x; 1.0135x over previous
"""Distributed single-head attention for TRN2 (8 NeuronCores).

Reference computation (per batch b):
    q = x @ Wq; k = x @ Wk; v = x @ Wv          (x: [S, E])
    s = (q @ k.T) / sqrt(DK) - 1e15 * mask
    out = softmax(s, axis=-1) @ v               ([S, DV])

Sharding: 8 cores = 4 batches x 2 sequence halves. Each core computes
attention for 1024 queries of one batch; K/V are recomputed per core from
the full sequence (softmax is permutation invariant over keys, so the
sequence is rotated per-core to put its queries first).

Host-prepared per-core layout:
  - xt  [E, S]  bf16: x_b^T, sequence permuted, queries first.
  - wq/wk/wv [P, EC*DK] bf16: packed [p, c*DK+d] = W[c*128+p, d]
                (wq pre-scaled by 1/sqrt(DK)).
  - mm_ [P, KTILES*SQ] fp8e4: the RAW mask ({0,1} exact in fp8 — halves
                mask DMA), keys permuted like xt, partition-major
                (mm_[p, t*SQ+q] = mask[key t*128+p, q]) for sequential DRAM.
  - pen [P, P] fp8e4: -30 * I. The mask penalty is applied INSIDE the
                score accumulation group: st = kt^T qt - 30*m, so
                exp(st) is already-masked P (exp(-30) ~ 5e-12 is
                negligible vs softmax sums ~1e3). This removes the wide
                DVE mask multiply — DVE ops pay a ~(dur-266ns) pipeline
                DRAIN before the next op can issue, so each [128,2048]
                op costs ~1.6us effective; DVE was the loop pacer.
  - out [DV, SQ] bf16: attention output transposed; host casts + transposes.

Schedule (v4):
  - PE warmup: dummy matmuls on ones right after the preamble trip the
    HAM activity monitor so the PE clock is 2.4 GHz when chunk 0 lands.
  - sync-queue DMAs (16 engines; scalar queue only has 4), first-use
    order: w, xa chunks (query-half columns), xb ~2 behind, masks last —
    same-queue FIFO delays mask traffic behind the x stream for free.
  - chunk loop c=0..7: qt0 qt1 k01 (xa), k23 (xb), vtA (xa) per chunk,
    into three [128,1024] PSUM pair-tiles + 2 vt banks. Wide PSUM->SBUF
    copies in first-use order: qt (ACT); k01, k23, vtA (DVE).
  - PSUM bank map is chosen so the score tiles alias the EARLIEST-copied
    projection banks: st pool (3 bufs, banks 2-7) reuses qt/k01/k23;
    the OT accumulators (2x [128,512]) reuse the vt banks 0-1 after the
    pre-loop vtB matmuls + copies. st bufs=3 decouples score(t+2) WAR
    from exp(t) so the exp stream never waits a just-issued score.
  - attention loop over PAIRS of key tiles: per tile, scores are 2 bf16
    matmuls + 2 fp8 penalty matmuls (same accumulation group), exp on
    ACT, then OT. DVE only accumulates the pair denominator (pairs 0-6).
  - denominator: ones[128,128]^T @ (folded pairs-0-6 sum, then pair-7 P)
    accumulated into an st-pool PSUM tile during pair 7 — rowsum(q)
    replicated across partitions; normalization is an elementwise
    multiply against the OT accumulators in native [dv, q] layout
    (reciprocal_approx_fast); the host transposes the [dv, q] result.
"""
import math
from contextlib import ExitStack

import ml_dtypes
import numpy as np

import concourse.bass as bass
import concourse.tile as tile
from concourse import bacc, bass_isa, masks, mybir
from concourse.bass_utils import run_bass_kernel_spmd

B, S, E, DK, DV = 4, 2048, 1024, 128, 128
SQ = S // 2  # queries per core
P = 128  # SBUF partitions
EC = E // P  # contraction chunks for projections
KTILES = S // P  # key tiles
N_WARM = 40  # PE warmup matmuls: bridge the PE from the preamble until
# the first x chunk lands (~11us) so the HAM clock gate never re-throttles
PEN = 30.0  # additive mask penalty (exact in fp8e4; exp(-30) ~ 5e-12)

f32 = mybir.dt.float32
bf16 = mybir.dt.bfloat16
f8e4 = mybir.dt.float8e4

# test.py pokes these to get profiling info
TRACE = False
LAST_RESULT = None


def build():
    nc = bacc.Bacc()
    xta = nc.declare_dram_parameter("xta", [E, SQ], bf16, isOutput=False)
    xtb = nc.declare_dram_parameter("xtb", [E, SQ], bf16, isOutput=False)
    wq = nc.declare_dram_parameter("wq", [P, EC * DK], bf16, isOutput=False)
    wk = nc.declare_dram_parameter("wk", [P, EC * DK], bf16, isOutput=False)
    wv = nc.declare_dram_parameter("wv", [P, EC * DV], bf16, isOutput=False)
    mm_ = nc.declare_dram_parameter("mm", [P, KTILES * SQ], f8e4, isOutput=False)
    pen = nc.declare_dram_parameter("pen", [P, P], f8e4, isOutput=False)
    out = nc.declare_dram_parameter("out", [P, SQ], bf16, isOutput=True)

    with ExitStack() as ctx:
        tc = ctx.enter_context(tile.TileContext(nc))
        const_pool = ctx.enter_context(tc.tile_pool(name="const", bufs=1))
        in_pool = ctx.enter_context(tc.tile_pool(name="inputs", bufs=1))
        proj_sb = ctx.enter_context(tc.tile_pool(name="proj", bufs=1))
        p_pool = ctx.enter_context(tc.tile_pool(name="p", bufs=6))
        stat = ctx.enter_context(tc.tile_pool(name="stat", bufs=1))

        # vt PSUM pool (banks 0-1): vtA -> vtB -> the two OT accumulators
        vt_ps = ctx.enter_context(tc.tile_pool(name="vt_ps", bufs=2, space="PSUM"))

        ones_mat = const_pool.tile([P, P], bf16)
        nc.gpsimd.memset(ones_mat[:], 1.0)
        # preload the exp table set off the critical path
        warm = const_pool.tile([1, 2], f32)
        nc.gpsimd.memset(warm[:], 0.0)
        nc.scalar.activation(warm[:], warm[:], mybir.ActivationFunctionType.Exp)

        # --- input loads: sync HWDGE queue, first-use order ---
        w_sb = {}
        for name in ("wq", "wk", "wv"):
            w_sb[name] = in_pool.tile([P, EC * DK], bf16, tag=name, name=f"w_{name}")
        pen_sb = in_pool.tile([P, P], f8e4, tag="pen", name="pen")
        xa_sb = []
        xb_sb = []
        for c in range(EC):
            xa_sb.append(in_pool.tile([P, SQ], bf16, tag=f"xa{c}", name=f"xa{c}"))
            xb_sb.append(in_pool.tile([P, SQ], bf16, tag=f"xb{c}", name=f"xb{c}"))
        m_all = [
            in_pool.tile([P, 4 * SQ], f8e4, tag=f"mq{qtr}", name=f"mq{qtr}")
            for qtr in range(4)
        ]

        # ALL xa chunks stream before any xb: the xa-only slabs
        # (qt/k01/vtA) then finish ~6us earlier, so the qt/k01 copies,
        # the first scores, and the exp stream all start that much
        # sooner. k23 (xb-only) runs as its own back-to-back PE stream
        # behind the xa one. mask quarter 0 slots in before xb6/xb7
        # (which only feed the chunk-7 k23 matmuls).
        # Single sync HWDGE queue (16 DMA engines — scalar's only gets 4,
        # and a parallel mask queue steals xa bandwidth). xta/xtb are
        # separate contiguous DRAM tensors so every transfer is a fully
        # sequential read. FIFO insertion points: mask quarter 0 after
        # xb4 (pen01 needs it right after the qt/k01 copies), the rest
        # behind the xb tail (their pen matmuls run pairs 0+).
        nc.sync.dma_start(w_sb["wq"][:], wq[:, :])
        nc.sync.dma_start(xa_sb[0][:], xta[0:P, :])
        nc.sync.dma_start(w_sb["wk"][:], wk[:, :])
        nc.sync.dma_start(w_sb["wv"][:], wv[:, :])
        nc.sync.dma_start(pen_sb[:], pen[:, :])
        for c in range(1, EC):
            nc.sync.dma_start(xa_sb[c][:], xta[c * P : (c + 1) * P, :])
        # mask quarters 0-1 ahead of the xb stream (their pen matmuls run
        # right after the qt/k01 copies); quarters 2-3 behind it (their
        # consumers run from loop pair 1 on)
        nc.sync.dma_start(m_all[0][:], mm_[:, 0 : 4 * SQ])
        nc.sync.dma_start(m_all[1][:], mm_[:, 4 * SQ : 8 * SQ])
        for c in range(EC):
            nc.sync.dma_start(xb_sb[c][:], xtb[c * P : (c + 1) * P, :])
        for qtr in range(2, 4):
            nc.sync.dma_start(
                m_all[qtr][:], mm_[:, qtr * 4 * SQ : (qtr + 1) * 4 * SQ]
            )

        def m_tile(t):  # mask columns for key tile t
            qtr, o = divmod(t, 4)
            return m_all[qtr][:, o * SQ : (o + 1) * SQ]

        # --- PE warmup: release the HAM clock gate before chunk 0 lands ---
        with tc.tile_pool(name="warm_ps", bufs=1, space="PSUM") as warm_pool:
            wps = warm_pool.tile([P, P], f32)
            for i in range(N_WARM):
                nc.tensor.matmul(
                    wps[:], ones_mat[:], ones_mat[:],
                    start=(i == 0), stop=(i == N_WARM - 1),
                )

        # running sum over PAIRS of masked P tiles (pairs 0..6)
        acc_d = stat.tile([P, 2 * SQ], bf16)

        # --- projections ---
        qt_sb = proj_sb.tile([P, SQ], bf16)
        kt_sb = proj_sb.tile([P, 4 * 512], bf16)  # 4 slabs, contiguous

        def kt_slab(s):
            return kt_sb[:, s * 512 : (s + 1) * 512]

        vt_sb = [
            proj_sb.tile([P, SQ], bf16, tag=f"vt{h}", name=f"vt{h}")
            for h in range(2)
        ]
        v_sb = [
            proj_sb.tile([P, 8 * DV], bf16, tag=f"v{h}", name=f"v{h}")
            for h in range(2)
        ]

        def v_tile(t):
            return v_sb[t // 8][:, (t % 8) * DV : (t % 8 + 1) * DV]

        def wslice(w, c):
            return w[:, c * DK : (c + 1) * DK]

        def xs(c, j):  # x chunk c, 512-column slab j (0..3)
            half = xa_sb if j < 2 else xb_sb
            jj = j % 2
            return half[c][:, jj * 512 : (jj + 1) * 512]

        ps_vt = {}
        for tag in ("vtA0", "vtA1"):
            ps_vt[tag] = vt_ps.tile([P, 512], f32, tag="vtps", name=f"ps_{tag}")

        proj_ctx = ExitStack()
        proj_ps = proj_ctx.enter_context(
            tc.tile_pool(name="proj_ps", bufs=2, space="PSUM")
        )
        ps_qt = proj_ps.tile([P, SQ], f32, tag="pps", name="ps_qt")
        ps_k01 = proj_ps.tile([P, SQ], f32, tag="pps", name="ps_k01")

        # xa-only stream: qt/k01/vtA accumulate over the xa chunks alone,
        # so they never wait on the trailing xb transfers. k23 (xb-only)
        # runs later through the vt banks, after the vtA copies.
        for c in range(EC):
            st_flags = dict(start=(c == 0), stop=(c == EC - 1))
            for j in range(2):  # QT (queries = columns 0..1023)
                nc.tensor.matmul(
                    ps_qt[:, j * 512 : (j + 1) * 512],
                    wslice(w_sb["wq"], c), xs(c, j), **st_flags,
                )
            for j in range(2):  # KT slabs 0-1 (keys 0..1023, xa)
                nc.tensor.matmul(
                    ps_k01[:, j * 512 : (j + 1) * 512],
                    wslice(w_sb["wk"], c), xs(c, j), **st_flags,
                )
            for j in range(2):  # VT slabs A0-A1 (keys 0..1023, xa)
                nc.tensor.matmul(
                    ps_vt[f"vtA{j}"][:], wslice(w_sb["wv"], c), xs(c, j), **st_flags
                )

        # wide PSUM -> SBUF copies, first-use order per engine
        nc.scalar.copy(qt_sb[:], ps_qt[:])
        nc.vector.tensor_copy(kt_sb[:, 0:1024], ps_k01[:])
        nc.vector.tensor_copy(vt_sb[0][:, 0:512], ps_vt["vtA0"][:])
        nc.vector.tensor_copy(vt_sb[0][:, 512:1024], ps_vt["vtA1"][:])

        # V transpose A: ONE grouped xbar op; out[k, t, dv] = vt[dv, t*128+k]
        nc.sync.dma_start_transpose(
            v_sb[0][:].rearrange("p (t d) -> p t d", t=8), vt_sb[0][:]
        )

        proj_ctx.close()  # qt/k01 banks (+warm's) -> st pool

        st_ctx = ctx.enter_context(ExitStack())
        st_ps = st_ctx.enter_context(
            tc.tile_pool(name="st_ps", bufs=3, space="PSUM")
        )

        st_tiles = []

        def score_mms(t):
            st = st_ps.tile([P, SQ], f32, tag="st", name=f"st{t}")
            st_tiles.append(st)
            s, o = divmod(t, 4)
            for j in range(2):
                nc.tensor.matmul(
                    st[:, j * 512 : (j + 1) * 512],
                    kt_slab(s)[:, o * P : (o + 1) * P],
                    qt_sb[:, j * 512 : (j + 1) * 512],
                    start=True,
                    stop=False,
                )
            for j in range(2):  # -30 * mask, same accumulation group (fp8)
                nc.tensor.matmul(
                    st[:, j * 512 : (j + 1) * 512],
                    pen_sb[:],
                    m_tile(t)[:, j * 512 : (j + 1) * 512],
                    start=False,
                    stop=True,
                )

        # PE order from here: sc/pen(0-2) -> k23 xb-stream -> sc/pen(3-5)
        # -> vtB xb-stream -> loop (lead 6). The exp stream starts as soon
        # as the qt/k01 copies + mask quarter 0 land, while the xb-side
        # work (paced by xb DMA-completion semaphores that lag the data
        # ~2.5us) fills the PE stream behind the pre-emitted scores.
        for t in range(3):
            score_mms(t)

        # k23 through the vt banks (WAR on the vtA copies)
        ps_k23 = [
            vt_ps.tile([P, 512], f32, tag="vtps", name=f"ps_k23{j}")
            for j in range(2)
        ]
        for c in range(EC):
            st_flags = dict(start=(c == 0), stop=(c == EC - 1))
            for j in range(2):  # KT slabs 2-3 (keys 1024..2047, xb)
                nc.tensor.matmul(
                    ps_k23[j][:], wslice(w_sb["wk"], c), xs(c, 2 + j), **st_flags
                )
        nc.vector.tensor_copy(kt_sb[:, 1024:1536], ps_k23[0][:])
        nc.vector.tensor_copy(kt_sb[:, 1536:2048], ps_k23[1][:])

        for t in range(3, 6):
            score_mms(t)

        # VT slabs B0/B1 on the vt banks (WAR on the k23 copies); their
        # copies + per-slab transposes are emitted inside loop pairs 1-2
        # so the DVE accumulates for pairs 0-1 aren't delayed.
        ps_vt["vtB0"] = vt_ps.tile([P, 512], f32, tag="vtps", name="ps_vtB0")
        ps_vt["vtB1"] = vt_ps.tile([P, 512], f32, tag="vtps", name="ps_vtB1")
        for c in range(EC):
            for j in range(2):
                nc.tensor.matmul(
                    ps_vt[f"vtB{j}"][:],
                    wslice(w_sb["wv"], c),
                    xs(c, 2 + j),
                    start=(c == 0),
                    stop=(c == EC - 1),
                )

        # OT accumulators on the vt banks (freed by the vtB copies)
        ot = [
            vt_ps.tile([P, 512], f32, tag="vtps", name=f"ot{j}") for j in range(2)
        ]

        rs = None  # denominator PSUM tile (st pool), created at pair 6
        last_pp = None

        for r in range(KTILES // 2):
            pp = p_pool.tile([P, 2 * SQ], bf16, tag="p", name=f"pp{r}")
            for h in range(2):
                t = 2 * r + h
                nc.scalar.activation(
                    pp[:, h * SQ : (h + 1) * SQ],
                    st_tiles[t][:],
                    mybir.ActivationFunctionType.Exp,
                )
            # accumulate pairs 0..6; pair 7 feeds the denominator matmuls
            if r == 0:
                nc.vector.tensor_copy(acc_d[:], pp[:])
            elif r < 7:
                nc.vector.tensor_add(acc_d[:], acc_d[:], pp[:])
            else:
                last_pp = pp
                # finish the denominator BEFORE the final OTs so the
                # recip -> normalize -> store tail starts earlier
                for h in range(2):
                    for j in range(2):
                        nc.tensor.matmul(
                            rs[:, j * 512 : (j + 1) * 512], ones_mat[:],
                            pp[:, h * SQ + j * 512 : h * SQ + (j + 1) * 512],
                            start=False, stop=(h == 1),
                        )
            for h in range(2):
                t = 2 * r + h
                if t + 6 < KTILES:
                    score_mms(t + 6)
                for j in range(2):
                    nc.tensor.matmul(
                        ot[j][:],
                        v_tile(t),
                        pp[:, h * SQ + j * 512 : h * SQ + (j + 1) * 512],
                        start=(t == 0),
                        stop=(t == KTILES - 1),
                    )
            if r == 1:  # vtB0 copy + transpose (v tiles 8-11), DVE stream
                nc.vector.tensor_copy(vt_sb[1][:, 0:512], ps_vt["vtB0"][:])
                nc.sync.dma_start_transpose(
                    v_sb[1][:, 0:512].rearrange("p (t d) -> p t d", t=4),
                    vt_sb[1][:, 0:512],
                )
            if r == 2:  # vtB1 copy + transpose (v tiles 12-15)
                nc.vector.tensor_copy(vt_sb[1][:, 512:1024], ps_vt["vtB1"][:])
                nc.sync.dma_start_transpose(
                    v_sb[1][:, 512:1024].rearrange("p (t d) -> p t d", t=4),
                    vt_sb[1][:, 512:1024],
                )
            if r == 6:
                # fold pairs 0..6 and start the denominator matmuls on an
                # st-pool tile (WAR: its bank tenant exp13 just ran)
                acc_f = stat.tile([P, SQ], bf16)
                nc.vector.tensor_add(
                    acc_f[:], acc_d[:, 0:SQ], acc_d[:, SQ : 2 * SQ]
                )
                rs = st_ps.tile([P, SQ], f32, tag="st", name="rs")
                for j in range(2):
                    nc.tensor.matmul(
                        rs[:, j * 512 : (j + 1) * 512], ones_mat[:],
                        acc_f[:, j * 512 : (j + 1) * 512],
                        start=True, stop=False,
                    )

        # --- epilogue: normalize in [dv, q] layout, store ---
        rcp_rep = stat.tile([P, SQ], f32)
        o_sb = stat.tile([P, SQ], bf16)
        for j in range(2):  # per-half pipeline: recip -> scale -> store
            sl = slice(j * 512, (j + 1) * 512)
            nc.vector.reciprocal_approx_fast(rcp_rep[:, sl], rs[:, sl])
            nc.vector.tensor_mul(o_sb[:, sl], ot[j][:], rcp_rep[:, sl])
            nc.sync.dma_start(out[:, sl], o_sb[:, sl])

    nc.compile()
    return nc


_NC_CACHE = None


def kernel(inputs, mask, Wq, Wk, Wv):
    global _NC_CACHE, LAST_RESULT
    inputs = np.asarray(inputs)
    mask = np.asarray(mask)
    bf = ml_dtypes.bfloat16
    f8 = ml_dtypes.float8_e4m3fn
    scale = np.float32(1.0 / math.sqrt(DK))

    def pack_w(w):  # [E, DK] -> [p, c*DK+d] = w[c*128+p, d]
        w = np.asarray(w).astype(bf)
        return np.ascontiguousarray(
            w.reshape(EC, P, DK).transpose(1, 0, 2).reshape(P, EC * DK)
        )

    wq_h = pack_w(np.asarray(Wq) * scale)
    wk_h = pack_w(Wk)
    wv_h = pack_w(Wv)
    pen_h = (-PEN * np.eye(P, dtype=np.float32)).astype(f8)

    if _NC_CACHE is None:
        _NC_CACHE = build()
    nc = _NC_CACHE

    m8 = mask.astype(f8)  # raw mask, {0,1} exact in fp8
    in_maps = []
    for core in range(8):
        b, h = divmod(core, 2)
        q0 = h * SQ
        idx = np.r_[q0:S, 0:q0]  # rotate so this core's queries come first
        xb = inputs[b]  # [S, E] f32
        xt_core = np.asarray(xb[idx].T).astype(bf)  # [E, S]
        xta_core = np.ascontiguousarray(xt_core[:, 0:SQ])
        xtb_core = np.ascontiguousarray(xt_core[:, SQ:S])
        mt = m8[b, q0 : q0 + SQ, :][:, idx].T  # [S, SQ] keys-major
        # partition-major swizzle: mm_core[p, t*SQ+q] = mt[t*128+p, q]
        mm_core = np.ascontiguousarray(
            mt.reshape(KTILES, P, SQ).transpose(1, 0, 2).reshape(P, KTILES * SQ)
        )
        in_maps.append(
            {
                "xta": xta_core, "xtb": xtb_core,
                "wq": wq_h, "wk": wk_h, "wv": wv_h,
                "mm": mm_core, "pen": pen_h,
            }
        )

    res = run_bass_kernel_spmd(nc, in_maps, list(range(8)), trace=TRACE)
    LAST_RESULT = res
    outp = np.empty((B, S, DV), np.float32)
    for core in range(8):
        b, h = divmod(core, 2)
        q0 = h * SQ
        o = np.asarray(res.results[core]["out"]).astype(np.float32)  # [DV, SQ]
        outp[b, q0 : q0 + SQ, :] = o.T
    return outp
